# revision 32
# baseline (speedup 1.0000x reference)
"""Trainium2 Bass kernel for CausalWanSelfAttention (frame-causal windowed
attention with QK-RMSNorm + RoPE), sharded over 8 NeuronCores.

ZERO-COLLECTIVE design: on this backend any collective costs ~2ms/iteration
(fixed rendezvous, nearly independent of payload -- measured with tiny
128-byte AllGathers), so instead of AllGather-ing K/V each core redundantly
computes K and V projections (+RMSNorm+RoPE) for ALL tokens (~570us extra
matmul), and computes Q/attention/O only for its own 1/8 of the queries
(frame-balanced interleave: core c owns tokens [f, c*T:(c+1)*T] of every
frame f).

Per-core structure:
  1. streaming K+V pass over all L tokens (512-column chunks): K-hat + RMS
     stats + RoPE -> k_loc in local DRAM [DIM, L] bf16 (channel-major);
     V -> v_loc in local DRAM [L, DIM] bf16 (token-major).
  2. Q pass on own S tokens (identical math) -> qrot [128, NH, S] in SBUF.
  3. attention per head-pair, two query rounds ([0,512) and [512,S)):
     keys chunked 128-wide per frame; scores land in a 4-bank PSUM tile
     (several score tiles slot-packed per bank when the query width is
     small) so exp is 1-2 ACT instructions per batch; softmax denominator
     via ones-matmuls accumulated in PSUM (z rows 0/32, double-buffered
     across rounds -- one live accumulation group per (bank, partition)
     at a time); per-head 1/Z applied by DVE with a partition-broadcast
     tile.  Emission is software-pipelined: scores(b+1) issue before
     PV/Z(b) so the tensor engine stays busy during exp(b).
  4. O projection -> out [S, DIM] fp32.
"""

import math
import sys
from contextlib import ExitStack

import numpy as np

if "/opt/trn_rl_repo" not in sys.path:
    sys.path.insert(0, "/opt/trn_rl_repo")

import ml_dtypes

BF16 = ml_dtypes.bfloat16
NC = 8   # cores
D = 128  # head dim
EPS = 1e-6


def _chunks(n, width):
    return [(g * width, min(n, (g + 1) * width))
            for g in range((n + width - 1) // width)]


_BUILD_CACHE = {}


def build_program(NH, F, T, allowed_kf, cap_waits=True, debug=False):
    """Build the SPMD Bass program (identical on all 8 cores).

    NH: heads; F: frames; T: own tokens per (core, frame);
    allowed_kf[qf] = key frames query-frame qf attends to (suffix q-set per
    key frame required -- true for causal masks).
    """
    key = (NH, F, T, tuple(tuple(a) for a in allowed_kf), cap_waits, debug)
    if key in _BUILD_CACHE:
        return _BUILD_CACHE[key]

    import concourse.bass as bass
    import concourse.mybir as mybir
    import concourse.tile as tile
    from concourse.mybir import ActivationFunctionType as AF

    dt = mybir.dt
    DIM = NH * D
    S = F * T                # own tokens per core
    FRAME = NC * T           # tokens per frame
    L = F * FRAME            # all tokens
    SLICE = 512              # out-channel slice for v/o projections
    NSL = DIM // SLICE
    TOKCH = _chunks(S, 128)  # token chunks for o projection
    KCOL = _chunks(L, 512)   # column chunks for the K/V streaming pass
    NKCH = (FRAME + 127) // 128  # 128-wide key chunks per frame

    # for each key frame kf: first query frame that attends to it
    first_qf = {}
    for kf in range(F):
        qs = [qf for qf in range(F) if kf in allowed_kf[qf]]
        assert qs, f"key frame {kf} unused"
        assert qs == list(range(qs[0], F)), "non-suffix q-set unsupported"
        first_qf[kf] = qs[0]

    nc = bass.Bass()

    # ---------------- I/O ----------------
    xT_d = nc.dram_tensor("xT", [DIM, L], dt.bfloat16, kind="ExternalInput")
    xq_d = nc.dram_tensor("xTq", [DIM, S], dt.bfloat16, kind="ExternalInput")
    w_d = {}
    for nm in ("wqT", "wkT", "wvT", "woT"):
        w_d[nm] = nc.dram_tensor(nm, [DIM, DIM], dt.bfloat16, kind="ExternalInput")
    # packed per-channel affine constants: bq|gq|bq*gq|bk|gk|bk*gk
    bias_d = nc.dram_tensor("bias_pack", [128, 6 * NH], dt.float32,
                            kind="ExternalInput")
    bv_d = nc.dram_tensor("bv_r", [1, DIM], dt.bfloat16, kind="ExternalInput")
    bo_d = nc.dram_tensor("bo_r", [1, DIM], dt.float32, kind="ExternalInput")
    angS_d = nc.dram_tensor("angS", [128, L], dt.float32, kind="ExternalInput")
    angC_d = nc.dram_tensor("angC", [128, L], dt.float32, kind="ExternalInput")
    aqS_d = nc.dram_tensor("aqS", [128, S], dt.float32, kind="ExternalInput")
    aqC_d = nc.dram_tensor("aqC", [128, S], dt.float32, kind="ExternalInput")
    out_d = nc.dram_tensor("out", [S, DIM], dt.float32, kind="ExternalOutput")
    if debug:
        kdbg_d = nc.dram_tensor("kdbg", [DIM, L], dt.bfloat16,
                                kind="ExternalOutput")
        vdbg_d = nc.dram_tensor("vdbg", [L, DIM], dt.bfloat16,
                                kind="ExternalOutput")
        qdbg_d = nc.dram_tensor("qdbg", [128, NH * S], dt.bfloat16,
                                kind="ExternalOutput")
        odbg_d = nc.dram_tensor("odbg", [128, NH * S], dt.bfloat16,
                                kind="ExternalOutput")
        zdbg_d = nc.dram_tensor("zdbg", [NH, S], dt.float32,
                                kind="ExternalOutput")

    inv_sqrt_d = 1.0 / math.sqrt(D)

    with tile.TileContext(nc) as tc, ExitStack() as ctx:
        dram = ctx.enter_context(tc.tile_pool(name="dram", bufs=1, space="DRAM"))
        v_loc = dram.tile([L, DIM], dt.bfloat16)
        k_loc = dram.tile([DIM, L], dt.bfloat16)
        rk_dram = dram.tile([1, L], dt.float32)

        const = ctx.enter_context(tc.tile_pool(name="const", bufs=1))
        resid = ctx.enter_context(tc.tile_pool(name="resid", bufs=1))

        ones_key = const.tile([128, 1], dt.bfloat16)
        nc.vector.memset(ones_key, 1.0)
        ones_row = const.tile([1, 128], dt.bfloat16)
        nc.vector.memset(ones_row, 1.0)
        eps_t = const.tile([128, 1], dt.float32)
        nc.vector.memset(eps_t, EPS)

        bias_sb = const.tile([128, 6 * NH], dt.float32)
        nc.sync.dma_start(out=bias_sb[:], in_=bias_d[:])
        bq_sb = bias_sb[:, 0 * NH:1 * NH]
        gq_sb = bias_sb[:, 1 * NH:2 * NH]
        bqgq_sb = bias_sb[:, 2 * NH:3 * NH]
        bk_sb = bias_sb[:, 3 * NH:4 * NH]
        gk_sb = bias_sb[:, 4 * NH:5 * NH]
        bkgk_sb = bias_sb[:, 5 * NH:6 * NH]
        bv_sb = const.tile([1, DIM], dt.bfloat16)
        nc.sync.dma_start(out=bv_sb[:], in_=bv_d[:])
        xT_view = xT_d[:].rearrange("(kc p) l -> p kc l", p=128)

        # ================= K + V streaming pass =================
        kv = ExitStack()
        wpool = kv.enter_context(tc.tile_pool(name="w_kv", bufs=1))
        xpool = kv.enter_context(tc.tile_pool(name="x_kv", bufs=2))
        apool = kv.enter_context(tc.tile_pool(name="ang", bufs=2))
        kpool = kv.enter_context(tc.tile_pool(name="khat", bufs=2))
        evkv = kv.enter_context(tc.tile_pool(name="ev_kv", bufs=4))
        rpool = kv.enter_context(tc.tile_pool(name="rope", bufs=2))
        ps_k = kv.enter_context(tc.tile_pool(name="ps_k", bufs=3, space="PSUM"))
        ps_v = kv.enter_context(tc.tile_pool(name="ps_v", bufs=3, space="PSUM"))
        ps_s = kv.enter_context(tc.tile_pool(name="ps_s", bufs=2, space="PSUM"))

        wk_sb = wpool.tile([128, NH, DIM], dt.bfloat16, name="wk_sb")
        nc.sync.dma_start(
            out=wk_sb[:], in_=w_d["wkT"][:].rearrange("(kc p) n -> p kc n", p=128))
        wv_sb = wpool.tile([128, NH, DIM], dt.bfloat16, name="wv_sb")
        nc.sync.dma_start(
            out=wv_sb[:], in_=w_d["wvT"][:].rearrange("(kc p) n -> p kc n", p=128))

        for (ca, cb) in KCOL:
            w = cb - ca
            x_c = xpool.tile([128, NH, 512], dt.bfloat16, tag="xc")
            nc.sync.dma_start(out=x_c[:, :, :w], in_=xT_view[:, :, ca:cb])
            ang = apool.tile([128, 2, 512], dt.float32, tag="ang")
            nc.sync.dma_start(out=ang[:, 0, :w], in_=angS_d[:, ca:cb])
            nc.sync.dma_start(out=ang[:, 1, :w], in_=angC_d[:, ca:cb])
            trig = apool.tile([128, 2, 512], dt.bfloat16, tag="trig")
            nc.scalar.activation(trig[:, :, :w], ang[:, :, :w], AF.Sin)

            khat_c = kpool.tile([128, NH, 512], dt.bfloat16, tag="khat")
            ss = ps_s.tile([1, 512], dt.float32, tag="ss")
            for m in range(NH):
                ps = ps_k.tile([128, 512], dt.float32, tag="psk")
                for kc in range(NH):
                    nc.tensor.matmul(ps[:, :w],
                                     wk_sb[:, kc, m * 128:(m + 1) * 128],
                                     x_c[:, kc, :w],
                                     start=(kc == 0), stop=(kc == NH - 1))
                sq = evkv.tile([128, 512], dt.bfloat16, tag="sq")
                nc.scalar.activation(sq[:, :w], ps[:, :w], AF.Square,
                                     bias=bk_sb[:, m:m + 1])
                nc.scalar.activation(khat_c[:, m, :w], ps[:, :w], AF.Identity,
                                     bias=bkgk_sb[:, m:m + 1],
                                     scale=gk_sb[:, m:m + 1])
                nc.tensor.matmul(ss[0:1, :w], ones_key[:], sq[:, :w],
                                 start=(m == 0), stop=(m == NH - 1))
            # r = 1/sqrt(mean+eps), broadcast to 128 partitions via DRAM
            rt = evkv.tile([1, 512], dt.float32, tag="rt")
            nc.scalar.activation(rt[0:1, :w], ss[0:1, :w], AF.Sqrt,
                                 bias=eps_t[0:1, :], scale=1.0 / DIM)
            nc.vector.reciprocal(rt[0:1, :w], rt[0:1, :w])
            nc.sync.dma_start(out=rk_dram[0:1, ca:cb], in_=rt[0:1, :w])
            rb = rpool.tile([128, 512], dt.float32, tag="rb")
            nc.sync.dma_start(
                out=rb[:, :w],
                in_=bass.AP(tensor=rk_dram.tensor,
                            offset=rk_dram[0:1, :].offset + ca,
                            ap=[[0, 128], [1, w]]))
            ct = rpool.tile([128, 512], dt.bfloat16, tag="ct")
            st = rpool.tile([128, 512], dt.bfloat16, tag="st")
            nc.vector.tensor_mul(ct[:, :w], trig[:, 1, :w], rb[:, :w])
            nc.vector.tensor_mul(st[:, :w], trig[:, 0, :w], rb[:, :w])
            for m in range(NH):
                sw = rpool.tile([128, 512], dt.bfloat16, tag="sw")
                nc.sync.dma_start(out=sw[0:64, :w], in_=khat_c[64:128, m, :w])
                nc.sync.dma_start(out=sw[64:128, :w], in_=khat_c[0:64, m, :w])
                t1 = rpool.tile([128, 512], dt.bfloat16, tag="t1")
                t2 = rpool.tile([128, 512], dt.bfloat16, tag="t2")
                t3 = rpool.tile([128, 512], dt.bfloat16, tag="t3")
                nc.vector.tensor_mul(t1[:, :w], khat_c[:, m, :w], ct[:, :w])
                nc.vector.tensor_mul(t2[:, :w], sw[:, :w], st[:, :w])
                nc.vector.tensor_add(t3[:, :w], t1[:, :w], t2[:, :w])
                nc.sync.dma_start(out=k_loc[m * 128:(m + 1) * 128, ca:cb],
                                  in_=t3[:, :w])
            # V for these tokens (token-major out)
            for sl in range(NSL):
                for (ra, rb_) in _chunks(w, 128):
                    rw = rb_ - ra
                    psv = ps_v.tile([128, SLICE], dt.float32, tag="psv")
                    for kc in range(NH):
                        nc.tensor.matmul(psv[:rw, :], x_c[:, kc, ra:rb_],
                                         wv_sb[:, kc, sl * SLICE:(sl + 1) * SLICE],
                                         start=(kc == 0), stop=False)
                    nc.tensor.matmul(psv[:rw, :], ones_row[0:1, :rw],
                                     bv_sb[0:1, sl * SLICE:(sl + 1) * SLICE],
                                     start=False, stop=True)
                    vt = evkv.tile([128, SLICE], dt.bfloat16, tag="vev")
                    nc.scalar.activation(vt[:rw, :], psv[:rw, :], AF.Copy)
                    nc.sync.dma_start(
                        out=v_loc[ca + ra:ca + rb_,
                                  sl * SLICE:(sl + 1) * SLICE],
                        in_=vt[:rw, :])
        kv.close()

        # ================= Q pass (own tokens) =================
        qrot = resid.tile([128, NH, S], dt.bfloat16, name="qrot")
        qp = ExitStack()
        wpool = qp.enter_context(tc.tile_pool(name="w_q", bufs=1))
        apool = qp.enter_context(tc.tile_pool(name="ang_q", bufs=1))
        qpool = qp.enter_context(tc.tile_pool(name="qhat", bufs=1))
        evq = qp.enter_context(tc.tile_pool(name="ev_q", bufs=4))
        rpool = qp.enter_context(tc.tile_pool(name="rope_q", bufs=3))
        ps_k = qp.enter_context(tc.tile_pool(name="ps_q", bufs=4, space="PSUM"))
        ps_s = qp.enter_context(tc.tile_pool(name="ps_sq", bufs=2, space="PSUM"))

        wq_sb = wpool.tile([128, NH, DIM], dt.bfloat16, name="wq_sb")
        nc.sync.dma_start(
            out=wq_sb[:], in_=w_d["wqT"][:].rearrange("(kc p) n -> p kc n", p=128))
        xq_sb = wpool.tile([128, NH, S], dt.bfloat16, name="xq_sb")
        nc.sync.dma_start(out=xq_sb[:],
                          in_=xq_d[:].rearrange("(kc p) s -> p kc s", p=128))
        aq = apool.tile([128, 2, S], dt.float32, name="aq")
        nc.sync.dma_start(out=aq[:, 0, :], in_=aqS_d[:])
        nc.sync.dma_start(out=aq[:, 1, :], in_=aqC_d[:])
        trigq = apool.tile([128, 2, S], dt.float32, name="trigq")
        nc.scalar.activation(trigq[:], aq[:], AF.Sin)
        qhat = qpool.tile([128, NH, S], dt.bfloat16, name="qhat")
        rq_dram = dram.tile([1, S], dt.float32, name="rq_dram")

        QH = _chunks(S, 512)  # [(0,512),(512,585)]
        ssq = {}
        for qi, (qa, qb) in enumerate(QH):
            ssq[qi] = ps_s.tile([1, 512], dt.float32, tag="ssq", name=f"ssq{qi}")
        for m in range(NH):
            ps = {}
            for qi, (qa, qb) in enumerate(QH):
                ps[qi] = ps_k.tile([128, 512], dt.float32, tag="psq", name=f"psq{qi}")
                for kc in range(NH):
                    nc.tensor.matmul(ps[qi][:, :qb - qa],
                                     wq_sb[:, kc, m * 128:(m + 1) * 128],
                                     xq_sb[:, kc, qa:qb],
                                     start=(kc == 0), stop=(kc == NH - 1))
                sq = evq.tile([128, 512], dt.bfloat16, tag="sqq")
                nc.scalar.activation(sq[:, :qb - qa], ps[qi][:, :qb - qa],
                                     AF.Square, bias=bq_sb[:, m:m + 1])
                nc.scalar.activation(qhat[:, m, qa:qb], ps[qi][:, :qb - qa],
                                     AF.Identity, bias=bqgq_sb[:, m:m + 1],
                                     scale=gq_sb[:, m:m + 1])
                nc.tensor.matmul(ssq[qi][0:1, :qb - qa], ones_key[:],
                                 sq[:, :qb - qa],
                                 start=(m == 0), stop=(m == NH - 1))
        for qi, (qa, qb) in enumerate(QH):
            rt = evq.tile([1, 512], dt.float32, tag="rtq")
            nc.scalar.activation(rt[0:1, :qb - qa], ssq[qi][0:1, :qb - qa],
                                 AF.Sqrt, bias=eps_t[0:1, :], scale=1.0 / DIM)
            nc.vector.reciprocal(rt[0:1, :qb - qa], rt[0:1, :qb - qa])
            nc.sync.dma_start(out=rq_dram[0:1, qa:qb], in_=rt[0:1, :qb - qa])
        rbq = rpool.tile([128, S], dt.float32, name="rbq")
        nc.sync.dma_start(
            out=rbq[:],
            in_=bass.AP(tensor=rq_dram.tensor, offset=rq_dram[0:1, :].offset,
                        ap=[[0, 128], [1, S]]))
        ctq = rpool.tile([128, S], dt.bfloat16, name="ctq")
        stq = rpool.tile([128, S], dt.bfloat16, name="stq")
        nc.vector.tensor_mul(ctq[:], trigq[:, 1, :], rbq[:])
        nc.vector.tensor_mul(stq[:], trigq[:, 0, :], rbq[:])
        for m in range(NH):
            sw = rpool.tile([128, S], dt.bfloat16, tag="swq")
            nc.sync.dma_start(out=sw[0:64, :], in_=qhat[64:128, m, :])
            nc.sync.dma_start(out=sw[64:128, :], in_=qhat[0:64, m, :])
            t1 = rpool.tile([128, S], dt.bfloat16, tag="t1q")
            t2 = rpool.tile([128, S], dt.bfloat16, tag="t2q")
            nc.vector.tensor_mul(t1[:], qhat[:, m, :], ctq[:])
            nc.vector.tensor_mul(t2[:], sw[:], stq[:])
            nc.vector.tensor_add(qrot[:, m, :], t1[:], t2[:])
        qp.close()

        # ================= attention =================
        # per head-pair pg: keys of frame kf are columns [kf*FRAME, ...) of
        # krot; V streamed from v_loc per (kf, chunk).  Scores land in a
        # 4-bank PSUM tile s_t; exp is one ACT op per batch of <=4 chunks.
        # o accumulates in PSUM (passA: q cols [0,512) -- 2 banks; passB:
        # [512,S) -- 1 shared bank), z via ones-matmuls into 1 bank.
        oT_sb = resid.tile([128, NH, S], dt.bfloat16, name="oT_sb")
        JB = 4  # PSUM banks per score tile
        PASSES = _chunks(S, 512)
        NPG = NH // 2
        att = ExitStack()
        att_k = att.enter_context(tc.tile_pool(name="att_k", bufs=2))
        att_v = att.enter_context(tc.tile_pool(name="att_v", bufs=2))
        att_s = att.enter_context(tc.tile_pool(name="att_s", bufs=1, space="PSUM"))
        att_o = att.enter_context(tc.tile_pool(name="att_o", bufs=1, space="PSUM"))
        att_z = att.enter_context(tc.tile_pool(name="att_z", bufs=1, space="PSUM"))
        att_p = att.enter_context(tc.tile_pool(name="att_p", bufs=3))
        att_m = att.enter_context(tc.tile_pool(name="att_m", bufs=2))

        PB = S - 512  # passB width

        for pg in range(NPG):
            kr_t = att_k.tile([128, 2, L], dt.bfloat16, tag="kr")
            nc.sync.dma_start(
                out=kr_t[:],
                in_=k_loc[pg * 256:(pg + 1) * 256, :]
                .rearrange("(hi p) l -> p hi l", p=128))
            # V chunk tiles for all (kf, ci), loaded once per pg
            v_t = {}
            for kf in range(F):
                for ci in range(NKCH):
                    ja, jb = 128 * ci, min(128 * (ci + 1), FRAME)
                    v_t[(kf, ci)] = att_v.tile([128, 256], dt.bfloat16,
                                               tag=f"v{kf}_{ci}",
                                               name=f"v{kf}_{ci}")
                    nc.sync.dma_start(
                        out=v_t[(kf, ci)][:jb - ja, :],
                        in_=v_loc[kf * FRAME + ja:kf * FRAME + jb,
                                  pg * 256:(pg + 1) * 256])

            oA = att_o.tile([128, 2, 512], dt.float32, name="oA")

            # two rounds over query ranges; PSUM o banks are reused across
            # rounds (disjoint group lifetimes -- a (bank, partition-row) can
            # only host one live accumulation group at a time); z double-
            # buffered across rounds
            for pi, (Qa, Qb) in enumerate(PASSES):
                QW = Qb - Qa
                z_t = att_z.tile([128, 512], dt.float32, tag="z", name="z_t")
                # batch geometry per key-frame: small query widths pack
                # several score tiles per PSUM bank so each batch carries
                # enough matmul work to hide the exp round-trip latency
                batches = []
                for kf in range(F):
                    qa = max(Qa, T * first_qf[kf])
                    if qa >= Qb:
                        continue
                    qw = Qb - qa
                    slotw = 128 if qw <= 128 else (256 if qw <= 256 else 512)
                    per_bank = 512 // slotw
                    cap = JB * per_bank
                    units = [(kf, ci, hi, qa)
                             for ci in range(NKCH) for hi in range(2)]
                    for i in range(0, len(units), cap):
                        batches.append((slotw, per_bank, units[i:i + cap]))

                o_first, o_last = {}, {}
                for bi, (slotw, per_bank, b) in enumerate(batches):
                    for j, (kf, ci, hi, qa) in enumerate(b):
                        o_first.setdefault(hi, (bi, j))
                        o_last[hi] = (bi, j)

                def slot_ap(tile_, j, per_bank, slotw, kw, qw):
                    return tile_[:kw, j // per_bank,
                                 (j % per_bank) * slotw:
                                 (j % per_bank) * slotw + qw]

                s_tiles = {}

                def scores(bi):
                    slotw, per_bank, b = batches[bi]
                    s_t = att_s.tile([128, JB, 512], dt.float32, tag="s",
                                     name="s_t")
                    s_tiles[bi] = s_t
                    for j, (kf, ci, hi, qa) in enumerate(b):
                        kw = min(128, FRAME - 128 * ci)
                        nc.tensor.matmul(
                            slot_ap(s_t, j, per_bank, slotw, kw, Qb - qa),
                            kr_t[:, hi,
                                 kf * FRAME + 128 * ci:kf * FRAME + 128 * ci + kw],
                            qrot[:, 2 * pg + hi, qa:Qb],
                            start=True, stop=True)

                def expev(bi):
                    slotw, per_bank, b = batches[bi]
                    kf, ci, hi, qa = b[0]
                    qw = Qb - qa
                    n = len(b)
                    s_t = s_tiles.pop(bi)
                    p_t = att_p.tile([128, JB, 512], dt.bfloat16, tag="p")
                    fb, rem = n // per_bank, n % per_bank
                    base_s, base_p = s_t[:], p_t[:]
                    if fb:
                        ap_s = bass.AP(tensor=base_s.tensor, offset=base_s.offset,
                                       ap=[base_s.ap[0], [512, fb],
                                           [slotw, per_bank], [1, qw]])
                        ap_p = bass.AP(tensor=base_p.tensor, offset=base_p.offset,
                                       ap=[base_p.ap[0], [512, fb],
                                           [slotw, per_bank], [1, qw]])
                        nc.scalar.activation(ap_p, ap_s, AF.Exp,
                                             scale=inv_sqrt_d)
                    if rem:
                        off = fb * 512
                        ap_s = bass.AP(tensor=base_s.tensor,
                                       offset=base_s.offset + off,
                                       ap=[base_s.ap[0], [slotw, rem], [1, qw]])
                        ap_p = bass.AP(tensor=base_p.tensor,
                                       offset=base_p.offset + off,
                                       ap=[base_p.ap[0], [slotw, rem], [1, qw]])
                        nc.scalar.activation(ap_p, ap_s, AF.Exp,
                                             scale=inv_sqrt_d)
                    return p_t

                def pv_z(bi, p_t):
                    slotw, per_bank, b = batches[bi]
                    for j, (kf, ci, hi, qa) in enumerate(b):
                        kw = min(128, FRAME - 128 * ci)
                        qw = Qb - qa
                        nc.tensor.matmul(
                            oA[:, hi, qa - Qa:Qb - Qa],
                            v_t[(kf, ci)][:kw, hi * 128:(hi + 1) * 128],
                            slot_ap(p_t, j, per_bank, slotw, kw, qw),
                            start=o_first[hi] == (bi, j),
                            stop=o_last[hi] == (bi, j))
                        nc.tensor.matmul(
                            z_t[32 * hi:32 * hi + 1, qa - Qa:Qb - Qa],
                            ones_key[:kw, :],
                            slot_ap(p_t, j, per_bank, slotw, kw, qw),
                            start=o_first[hi] == (bi, j),
                            stop=o_last[hi] == (bi, j))

                prev = None
                prev_p = None
                for bi in range(len(batches)):
                    if prev is not None:
                        prev_p = expev(prev)
                    scores(bi)
                    if prev is not None:
                        pv_z(prev, prev_p)
                    prev = bi
                prev_p = expev(prev)
                pv_z(prev, prev_p)

                # 1/Z + eviction for this round
                for hi in range(2):
                    hh = 2 * pg + hi
                    zr = 32 * hi
                    z_sb = att_m.tile([128, 512], dt.float32, tag="zsb",
                                      name="zsb")
                    z_dram = dram.tile([1, 512], dt.float32, tag="zdram",
                                       bufs=2, name="zdram")
                    nc.scalar.activation(z_sb[zr:zr + 1, :QW],
                                         z_t[zr:zr + 1, :QW], AF.Copy)
                    nc.vector.reciprocal(z_sb[zr:zr + 1, :QW],
                                         z_sb[zr:zr + 1, :QW])
                    nc.sync.dma_start(out=z_dram[0:1, :QW],
                                      in_=z_sb[zr:zr + 1, :QW])
                    izb = att_m.tile([128, 512], dt.float32, tag="izb",
                                     name="izb")
                    nc.sync.dma_start(
                        out=izb[:, :QW],
                        in_=bass.AP(tensor=z_dram.tensor,
                                    offset=z_dram[0:1, :].offset,
                                    ap=[[0, 128], [1, QW]]))
                    if debug:
                        nc.sync.dma_start(out=zdbg_d[hh:hh + 1, Qa:Qb],
                                          in_=izb[0:1, :QW])
                    nc.vector.tensor_mul(oT_sb[:, hh, Qa:Qb],
                                         oA[:, hi, :QW], izb[:, :QW])
        att.close()
        if debug:
            nc.sync.dma_start(out=kdbg_d[:], in_=k_loc[:])
            nc.sync.dma_start(out=vdbg_d[:], in_=v_loc[:])
            nc.sync.dma_start(out=qdbg_d[:],
                              in_=qrot[:].rearrange("p m s -> p (m s)"))
            nc.sync.dma_start(out=odbg_d[:],
                              in_=oT_sb[:].rearrange("p m s -> p (m s)"))

        # ---------------- O projection ----------------
        bo_bc = const.tile([128, DIM], dt.float32)
        nc.sync.dma_start(
            out=bo_bc[:],
            in_=bass.AP(tensor=bo_d[:].tensor, offset=bo_d[:].offset,
                        ap=[[0, 128]] + bo_d[:].ap[1:]),
        )
        wpool = ctx.enter_context(tc.tile_pool(name="w_o", bufs=3))
        pspool = ctx.enter_context(
            tc.tile_pool(name="ps_o", bufs=len(TOKCH) + 1, space="PSUM"))
        evpool = ctx.enter_context(tc.tile_pool(name="ev_o", bufs=3))
        for sl in range(NSL):
            ps = {}
            for ti in range(len(TOKCH)):
                ps[ti] = pspool.tile([128, SLICE], dt.float32, tag="ops",
                                     name=f"ops{ti}")
            for m in range(NH):
                wt = wpool.tile([128, SLICE], dt.bfloat16, tag="wo")
                nc.sync.dma_start(
                    out=wt[:],
                    in_=w_d["woT"][m * 128:(m + 1) * 128,
                                   sl * SLICE:(sl + 1) * SLICE])
                for ti, (ta, tb) in enumerate(TOKCH):
                    nc.tensor.matmul(ps[ti][:tb - ta, :], oT_sb[:, m, ta:tb],
                                     wt[:], start=(m == 0), stop=(m == NH - 1))
            for ti, (ta, tb) in enumerate(TOKCH):
                tw = tb - ta
                ot = evpool.tile([128, SLICE], dt.float32, tag="oev")
                nc.vector.tensor_add(ot[:tw, :], ps[ti][:tw, :],
                                     bo_bc[:tw, sl * SLICE:(sl + 1) * SLICE])
                nc.sync.dma_start(
                    out=out_d[ta:tb, sl * SLICE:(sl + 1) * SLICE],
                    in_=ot[:tw, :])

    if cap_waits:
        _cap_sync_waits(nc, mybir)
    _BUILD_CACHE[key] = nc
    return nc


def _cap_sync_waits(nc, mybir, cap=1):
    """Walrus engine-instruction structs only have a limited number of sync
    wait slots.  Hoist excess waits onto InstNoOp carriers placed immediately
    before the instruction on the same engine stream."""
    exempt = (mybir.InstNoOp, mybir.InstEventSemaphore,
              mybir.InstAllEngineBarrier)
    for f in nc.m.functions:
        for bb in f.blocks:
            out = []
            changed = False
            for inst in bb.instructions:
                si = inst.sync_info
                if (si is None or len(si.on_wait) <= cap
                        or isinstance(inst, exempt)):
                    out.append(inst)
                    continue
                waits = list(si.on_wait)
                keep, excess = waits[:cap], waits[cap:]
                while excess:
                    batch, excess = excess[:cap], excess[cap:]
                    out.append(mybir.InstNoOp(
                        name=f"{inst.name}-w{len(out)}",
                        engine=inst.engine,
                        bass_nofuse=True,
                        sync_info=mybir.SyncInfo(on_wait=batch, on_update=[]),
                    ))
                inst.sync_info = mybir.SyncInfo(on_wait=keep,
                                                on_update=list(si.on_update))
                out.append(inst)
                changed = True
            if changed:
                bb.instructions = out


# ---------------------------------------------------------------------------
# host side
# ---------------------------------------------------------------------------
def _perm(NH):
    p = np.empty(NH * D, np.int64)
    for hh in range(NH):
        base = hh * D
        for j in range(D // 2):
            p[base + j] = base + 2 * j
            p[base + D // 2 + j] = base + 2 * j + 1
    return p


def _angles(freqs, idx, FRAME, w, start_frame, c0, c1, c):
    """Phase-shifted RoPE angle tables [128, len(idx)] for tokens `idx`."""
    fr = idx // FRAME
    rem = idx % FRAME
    hh_i = rem // w
    ww_i = rem % w
    n = len(idx)
    ang = np.empty((c, n), np.float32)
    ang[:c0, :] = freqs[start_frame + fr][:, :c0].T
    ang[c0:c0 + c1, :] = freqs[hh_i][:, c0:c0 + c1].T
    ang[c0 + c1:, :] = freqs[ww_i][:, c0 + c1:c].T

    def wrap(a):
        a = np.asarray(a, np.float64)
        return (a - 2 * np.pi * np.round(a / (2 * np.pi))).astype(np.float32)

    # top half encodes -sin via the (ang + pi) phase shift
    angS = np.ascontiguousarray(
        np.concatenate([wrap(ang + np.pi), wrap(ang)], 0), np.float32)
    angC = np.ascontiguousarray(
        np.concatenate([wrap(ang + np.pi / 2), wrap(ang + np.pi / 2)], 0),
        np.float32)
    return angS, angC


def _host_inputs(x, freqs, Wq, bq, Wk, bk, Wv, bv, Wo, bo, gq, gk,
                 f, h, w, num_heads, local_attn_size, sink_size, start_frame):
    NH = num_heads
    DIM = NH * D
    FRAME = h * w
    assert FRAME % NC == 0
    T = FRAME // NC
    S = f * T
    L = f * FRAME
    perm = _perm(NH)

    def bf(a):
        return np.ascontiguousarray(a, dtype=np.float32).astype(BF16)

    wqT = bf(Wq[perm].T)
    wkT = bf(Wk[perm].T)
    wvT = bf(Wv.T)
    woT = bf(Wo.T)

    def chunkmajor(a):
        return np.asarray(a, np.float32)[perm].reshape(NH, D).T
    bias_pack = np.ascontiguousarray(np.concatenate(
        [chunkmajor(bq), chunkmajor(gq), chunkmajor(bq) * chunkmajor(gq),
         chunkmajor(bk), chunkmajor(gk), chunkmajor(bk) * chunkmajor(gk)],
        axis=1), np.float32)
    bv_r = bf(bv.reshape(1, DIM))
    bo_r = np.ascontiguousarray(bo.reshape(1, DIM), np.float32)

    c = D // 2
    c1 = c // 3
    c0 = c - 2 * c1
    freqs = np.asarray(freqs, np.float32)

    xT_full = bf(np.asarray(x[0], np.float32).T)
    all_idx = np.arange(L)
    angS, angC = _angles(freqs, all_idx, FRAME, w, start_frame, c0, c1, c)

    in_maps = []
    tok_idx = []
    for core in range(NC):
        idx = np.concatenate(
            [fr * FRAME + T * core + np.arange(T) for fr in range(f)])
        tok_idx.append(idx)
        xTq = bf(np.asarray(x[0], np.float32)[idx].T)
        aqS, aqC = _angles(freqs, idx, FRAME, w, start_frame, c0, c1, c)
        in_maps.append({
            "xT": xT_full, "xTq": xTq,
            "wqT": wqT, "wkT": wkT, "wvT": wvT, "woT": woT,
            "bias_pack": bias_pack,
            "bv_r": bv_r, "bo_r": bo_r,
            "angS": angS, "angC": angC, "aqS": aqS, "aqC": aqC,
        })
    return in_maps, tok_idx, T, S


def _allowed(f, local_attn_size, sink_size):
    return [
        [kf for kf in range(f)
         if kf <= qf and (qf - kf < local_attn_size or kf < sink_size)]
        for qf in range(f)
    ]


def kernel(x, freqs, Wq, bq, Wk, bk, Wv, bv, Wo, bo, gq, gk,
           f, h, w, num_heads, local_attn_size, sink_size, start_frame,
           _trace=False):
    from concourse.bass_utils import run_bass_kernel_spmd

    f = int(f); h = int(h); w = int(w)
    num_heads = int(num_heads)
    local_attn_size = int(local_attn_size)
    sink_size = int(sink_size)
    start_frame = int(start_frame)

    x = np.asarray(x)
    B, L, DIM = x.shape
    assert B == 1 and DIM == num_heads * D

    allowed = _allowed(f, local_attn_size, sink_size)
    in_maps, tok_idx, T, S = _host_inputs(
        x, freqs, Wq, bq, Wk, bk, Wv, bv, Wo, bo, gq, gk,
        f, h, w, num_heads, local_attn_size, sink_size, start_frame)
    nc = build_program(num_heads, f, T, allowed)
    res = run_bass_kernel_spmd(nc, in_maps, core_ids=list(range(NC)),
                               trace=_trace)
    out = np.empty((1, L, DIM), np.float32)
    for core in range(NC):
        out[0, tok_idx[core]] = res.results[core]["out"]
    if _trace:
        kernel._last_results = res
    return out


# revision 37
# speedup vs baseline: 2.0623x; 2.0623x over previous
"""Trainium2 Bass kernel for CausalWanSelfAttention (frame-causal windowed
attention with QK-RMSNorm + RoPE), sharded over 8 NeuronCores.

ZERO-COLLECTIVE design: on this backend any collective costs ~2ms/iteration
(fixed rendezvous, nearly independent of payload -- measured with tiny
128-byte AllGathers), so instead of AllGather-ing K/V each core redundantly
computes K and V projections (+RMSNorm+RoPE) for ALL tokens (~570us extra
matmul), and computes Q/attention/O only for its own 1/8 of the queries
(frame-balanced interleave: core c owns tokens [f, c*T:(c+1)*T] of every
frame f).

Per-core structure:
  1. streaming K+V pass over all L tokens (512-column chunks): K-hat + RMS
     stats + RoPE -> k_loc in local DRAM [DIM, L] bf16 (channel-major);
     V -> v_loc in local DRAM [L, DIM] bf16 (token-major).
  2. Q pass on own S tokens (identical math) -> qrot [128, NH, S] in SBUF.
  3. attention per head-pair, two query rounds ([0,512) and [512,S)):
     keys chunked 128-wide per frame; scores land in a 4-bank PSUM tile
     (several score tiles slot-packed per bank when the query width is
     small) so exp is 1-2 ACT instructions per batch; softmax denominator
     via ones-matmuls accumulated in PSUM (z rows 0/32, double-buffered
     across rounds -- one live accumulation group per (bank, partition)
     at a time); per-head 1/Z applied by DVE with a partition-broadcast
     tile.  Emission is software-pipelined: scores(b+1) issue before
     PV/Z(b) so the tensor engine stays busy during exp(b).
  4. O projection -> out [S, DIM] fp32.
"""

import math
import sys
from contextlib import ExitStack

import numpy as np

if "/opt/trn_rl_repo" not in sys.path:
    sys.path.insert(0, "/opt/trn_rl_repo")

import ml_dtypes

BF16 = ml_dtypes.bfloat16
NC = 8   # cores
D = 128  # head dim
EPS = 1e-6


def _chunks(n, width):
    return [(g * width, min(n, (g + 1) * width))
            for g in range((n + width - 1) // width)]


_BUILD_CACHE = {}


def build_program(NH, F, T, allowed_kf, cap_waits=True, debug=False):
    """Build the SPMD Bass program (identical on all 8 cores).

    NH: heads; F: frames; T: own tokens per (core, frame);
    allowed_kf[qf] = key frames query-frame qf attends to (suffix q-set per
    key frame required -- true for causal masks).
    """
    key = (NH, F, T, tuple(tuple(a) for a in allowed_kf), cap_waits, debug)
    if key in _BUILD_CACHE:
        return _BUILD_CACHE[key]

    import concourse.bass as bass
    import concourse.mybir as mybir
    import concourse.tile as tile
    from concourse.mybir import ActivationFunctionType as AF

    dt = mybir.dt
    DIM = NH * D
    S = F * T                # own tokens per core
    FRAME = NC * T           # tokens per frame
    L = F * FRAME            # all tokens
    SLICE = 512              # out-channel slice for v/o projections
    NSL = DIM // SLICE
    TOKCH = _chunks(S, 128)  # token chunks for o projection
    KCOL = _chunks(L, 512)   # column chunks for the K/V streaming pass
    NKCH = (FRAME + 127) // 128  # 128-wide key chunks per frame

    # for each key frame kf: first query frame that attends to it
    first_qf = {}
    for kf in range(F):
        qs = [qf for qf in range(F) if kf in allowed_kf[qf]]
        assert qs, f"key frame {kf} unused"
        assert qs == list(range(qs[0], F)), "non-suffix q-set unsupported"
        first_qf[kf] = qs[0]

    nc = bass.Bass()

    # ---------------- I/O ----------------
    xT_d = nc.dram_tensor("xT", [DIM, L], dt.bfloat16, kind="ExternalInput")
    xq_d = nc.dram_tensor("xTq", [DIM, S], dt.bfloat16, kind="ExternalInput")
    w_d = {}
    for nm in ("wqT", "wkT", "wvT", "woT"):
        w_d[nm] = nc.dram_tensor(nm, [DIM, DIM], dt.bfloat16, kind="ExternalInput")
    # packed per-channel affine constants: bq|gq|bq*gq|bk|gk|bk*gk
    bias_d = nc.dram_tensor("bias_pack", [128, 6 * NH], dt.float32,
                            kind="ExternalInput")
    bv_d = nc.dram_tensor("bv_r", [1, DIM], dt.bfloat16, kind="ExternalInput")
    bo_d = nc.dram_tensor("bo_r", [1, DIM], dt.float32, kind="ExternalInput")
    angS_d = nc.dram_tensor("angS", [128, L], dt.float32, kind="ExternalInput")
    angC_d = nc.dram_tensor("angC", [128, L], dt.float32, kind="ExternalInput")
    aqS_d = nc.dram_tensor("aqS", [128, S], dt.float32, kind="ExternalInput")
    aqC_d = nc.dram_tensor("aqC", [128, S], dt.float32, kind="ExternalInput")
    out_d = nc.dram_tensor("out", [S, DIM], dt.float32, kind="ExternalOutput")
    if debug:
        kdbg_d = nc.dram_tensor("kdbg", [DIM, L], dt.bfloat16,
                                kind="ExternalOutput")
        vdbg_d = nc.dram_tensor("vdbg", [L, DIM], dt.bfloat16,
                                kind="ExternalOutput")
        qdbg_d = nc.dram_tensor("qdbg", [128, NH * S], dt.bfloat16,
                                kind="ExternalOutput")
        odbg_d = nc.dram_tensor("odbg", [128, NH * S], dt.bfloat16,
                                kind="ExternalOutput")
        zdbg_d = nc.dram_tensor("zdbg", [NH, S], dt.float32,
                                kind="ExternalOutput")

    inv_sqrt_d = 1.0 / math.sqrt(D)

    with tile.TileContext(nc) as tc, ExitStack() as ctx:
        dram = ctx.enter_context(tc.tile_pool(name="dram", bufs=1, space="DRAM"))
        v_loc = dram.tile([L, DIM], dt.bfloat16)
        k_loc = dram.tile([DIM, L], dt.bfloat16)
        rk_dram = dram.tile([1, L], dt.float32)

        const = ctx.enter_context(tc.tile_pool(name="const", bufs=1))
        resid = ctx.enter_context(tc.tile_pool(name="resid", bufs=1))

        ones_key = const.tile([128, 1], dt.bfloat16)
        nc.vector.memset(ones_key, 1.0)
        ones_row = const.tile([1, 128], dt.bfloat16)
        nc.vector.memset(ones_row, 1.0)
        eps_t = const.tile([128, 1], dt.float32)
        nc.vector.memset(eps_t, EPS)
        ones_f32 = const.tile([64, 128], dt.float32)
        nc.vector.memset(ones_f32, 1.0)

        bias_sb = const.tile([128, 6 * NH], dt.float32)
        nc.sync.dma_start(out=bias_sb[:], in_=bias_d[:])
        bq_sb = bias_sb[:, 0 * NH:1 * NH]
        gq_sb = bias_sb[:, 1 * NH:2 * NH]
        bqgq_sb = bias_sb[:, 2 * NH:3 * NH]
        bk_sb = bias_sb[:, 3 * NH:4 * NH]
        gk_sb = bias_sb[:, 4 * NH:5 * NH]
        bkgk_sb = bias_sb[:, 5 * NH:6 * NH]
        bv_sb = const.tile([1, DIM], dt.bfloat16)
        nc.sync.dma_start(out=bv_sb[:], in_=bv_d[:])
        xT_view = xT_d[:].rearrange("(kc p) l -> p kc l", p=128)

        # ================= K + V streaming pass =================
        kv = ExitStack()
        wpool = kv.enter_context(tc.tile_pool(name="w_kv", bufs=1))
        xpool = kv.enter_context(tc.tile_pool(name="x_kv", bufs=2))
        apool = kv.enter_context(tc.tile_pool(name="ang", bufs=2))
        kpool = kv.enter_context(tc.tile_pool(name="khat", bufs=2))
        evkv = kv.enter_context(tc.tile_pool(name="ev_kv", bufs=4))
        rpool = kv.enter_context(tc.tile_pool(name="rope", bufs=2))
        ps_k = kv.enter_context(tc.tile_pool(name="ps_k", bufs=3, space="PSUM"))
        ps_v = kv.enter_context(tc.tile_pool(name="ps_v", bufs=3, space="PSUM"))
        ps_s = kv.enter_context(tc.tile_pool(name="ps_s", bufs=2, space="PSUM"))

        wk_sb = wpool.tile([128, NH, DIM], dt.bfloat16, name="wk_sb")
        nc.sync.dma_start(
            out=wk_sb[:], in_=w_d["wkT"][:].rearrange("(kc p) n -> p kc n", p=128))
        wv_sb = wpool.tile([128, NH, DIM], dt.bfloat16, name="wv_sb")
        nc.sync.dma_start(
            out=wv_sb[:], in_=w_d["wvT"][:].rearrange("(kc p) n -> p kc n", p=128))

        for (ca, cb) in KCOL:
            w = cb - ca
            x_c = xpool.tile([128, NH, 512], dt.bfloat16, tag="xc")
            nc.sync.dma_start(out=x_c[:, :, :w], in_=xT_view[:, :, ca:cb])
            ang = apool.tile([128, 2, 512], dt.float32, tag="ang")
            nc.sync.dma_start(out=ang[:, 0, :w], in_=angS_d[:, ca:cb])
            nc.sync.dma_start(out=ang[:, 1, :w], in_=angC_d[:, ca:cb])
            trig = apool.tile([128, 2, 512], dt.bfloat16, tag="trig")
            nc.scalar.activation(trig[:, :, :w], ang[:, :, :w], AF.Sin)

            khat_c = kpool.tile([128, NH, 512], dt.bfloat16, tag="khat")
            ss = ps_s.tile([1, 512], dt.float32, tag="ss")
            for m in range(NH):
                ps = ps_k.tile([128, 512], dt.float32, tag="psk")
                for kc in range(NH):
                    nc.tensor.matmul(ps[:, :w],
                                     wk_sb[:, kc, m * 128:(m + 1) * 128],
                                     x_c[:, kc, :w],
                                     start=(kc == 0), stop=(kc == NH - 1))
                sq = evkv.tile([128, 512], dt.bfloat16, tag="sq")
                nc.scalar.activation(sq[:, :w], ps[:, :w], AF.Square,
                                     bias=bk_sb[:, m:m + 1])
                nc.scalar.activation(khat_c[:, m, :w], ps[:, :w], AF.Identity,
                                     bias=bkgk_sb[:, m:m + 1],
                                     scale=gk_sb[:, m:m + 1])
                nc.tensor.matmul(ss[0:1, :w], ones_key[:], sq[:, :w],
                                 start=(m == 0), stop=(m == NH - 1))
            # r = 1/sqrt(mean+eps), broadcast to 128 partitions via DRAM
            rt = evkv.tile([1, 512], dt.float32, tag="rt")
            nc.scalar.activation(rt[0:1, :w], ss[0:1, :w], AF.Sqrt,
                                 bias=eps_t[0:1, :], scale=1.0 / DIM)
            nc.vector.reciprocal(rt[0:1, :w], rt[0:1, :w])
            nc.sync.dma_start(out=rk_dram[0:1, ca:cb], in_=rt[0:1, :w])
            rb = rpool.tile([128, 512], dt.float32, tag="rb")
            nc.sync.dma_start(
                out=rb[:, :w],
                in_=bass.AP(tensor=rk_dram.tensor,
                            offset=rk_dram[0:1, :].offset + ca,
                            ap=[[0, 128], [1, w]]))
            ct = rpool.tile([128, 512], dt.bfloat16, tag="ct")
            st = rpool.tile([128, 512], dt.bfloat16, tag="st")
            nc.vector.tensor_mul(ct[:, :w], trig[:, 1, :w], rb[:, :w])
            nc.vector.tensor_mul(st[:, :w], trig[:, 0, :w], rb[:, :w])
            for m in range(NH):
                sw = rpool.tile([128, 512], dt.bfloat16, tag="sw")
                nc.sync.dma_start(out=sw[0:64, :w], in_=khat_c[64:128, m, :w])
                nc.sync.dma_start(out=sw[64:128, :w], in_=khat_c[0:64, m, :w])
                t1 = rpool.tile([128, 512], dt.bfloat16, tag="t1")
                t2 = rpool.tile([128, 512], dt.bfloat16, tag="t2")
                t3 = rpool.tile([128, 512], dt.bfloat16, tag="t3")
                nc.vector.tensor_mul(t1[:, :w], khat_c[:, m, :w], ct[:, :w])
                nc.vector.tensor_mul(t2[:, :w], sw[:, :w], st[:, :w])
                nc.vector.tensor_add(t3[:, :w], t1[:, :w], t2[:, :w])
                nc.sync.dma_start(out=k_loc[m * 128:(m + 1) * 128, ca:cb],
                                  in_=t3[:, :w])
            # V for these tokens (token-major out)
            for sl in range(NSL):
                for (ra, rb_) in _chunks(w, 128):
                    rw = rb_ - ra
                    psv = ps_v.tile([128, SLICE], dt.float32, tag="psv")
                    for kc in range(NH):
                        nc.tensor.matmul(psv[:rw, :], x_c[:, kc, ra:rb_],
                                         wv_sb[:, kc, sl * SLICE:(sl + 1) * SLICE],
                                         start=(kc == 0), stop=False)
                    nc.tensor.matmul(psv[:rw, :], ones_row[0:1, :rw],
                                     bv_sb[0:1, sl * SLICE:(sl + 1) * SLICE],
                                     start=False, stop=True)
                    vt = evkv.tile([128, SLICE], dt.bfloat16, tag="vev")
                    nc.scalar.activation(vt[:rw, :], psv[:rw, :], AF.Copy)
                    nc.sync.dma_start(
                        out=v_loc[ca + ra:ca + rb_,
                                  sl * SLICE:(sl + 1) * SLICE],
                        in_=vt[:rw, :])
        kv.close()

        # attention K/V prefetch: pools created now so pg 0/1 tiles start
        # streaming from k_loc/v_loc while the Q pass computes (kr DMAs are
        # chunk-split so each waits only on its own KV-pass chunk)
        att = ExitStack()
        att_k = att.enter_context(tc.tile_pool(name="att_k", bufs=2))
        att_v = att.enter_context(tc.tile_pool(name="att_v", bufs=2))
        NPG = NH // 2
        kr_cache = {}

        def load_pg(pg):
            if pg in kr_cache or pg >= NPG:
                return
            kr_t = att_k.tile([128, 2, L], dt.bfloat16, tag="kr")
            for (ca, cb) in KCOL:
                nc.sync.dma_start(
                    out=kr_t[:, :, ca:cb],
                    in_=k_loc[pg * 256:(pg + 1) * 256, ca:cb]
                    .rearrange("(hi p) l -> p hi l", p=128))
            v_t = {}
            for kf in range(F):
                for ci in range(NKCH):
                    ja, jb = 128 * ci, min(128 * (ci + 1), FRAME)
                    v_t[(kf, ci)] = att_v.tile([128, 256], dt.bfloat16,
                                               tag=f"v{kf}_{ci}",
                                               name=f"v{kf}_{ci}")
                    nc.sync.dma_start(
                        out=v_t[(kf, ci)][:jb - ja, :],
                        in_=v_loc[kf * FRAME + ja:kf * FRAME + jb,
                                  pg * 256:(pg + 1) * 256])
            kr_cache[pg] = (kr_t, v_t)

        load_pg(0)

        # ================= Q pass (own tokens) =================
        qrot = resid.tile([128, NH, S], dt.bfloat16, name="qrot")
        qp = ExitStack()
        wpool = qp.enter_context(tc.tile_pool(name="w_q", bufs=1))
        apool = qp.enter_context(tc.tile_pool(name="ang_q", bufs=1))
        qpool = qp.enter_context(tc.tile_pool(name="qhat", bufs=1))
        evq = qp.enter_context(tc.tile_pool(name="ev_q", bufs=3))
        rpool = qp.enter_context(tc.tile_pool(name="rope_q", bufs=2))
        ps_k = qp.enter_context(tc.tile_pool(name="ps_q", bufs=4, space="PSUM"))
        ps_s = qp.enter_context(tc.tile_pool(name="ps_sq", bufs=2, space="PSUM"))

        xq_sb = wpool.tile([128, NH, S], dt.bfloat16, name="xq_sb")
        nc.sync.dma_start(out=xq_sb[:],
                          in_=xq_d[:].rearrange("(kc p) s -> p kc s", p=128))
        aq = apool.tile([128, 2, S], dt.float32, name="aq")
        nc.sync.dma_start(out=aq[:, 0, :], in_=aqS_d[:])
        nc.sync.dma_start(out=aq[:, 1, :], in_=aqC_d[:])
        trigq = apool.tile([128, 2, S], dt.bfloat16, name="trigq")
        nc.scalar.activation(trigq[:], aq[:], AF.Sin)
        qhat = qpool.tile([128, NH, S], dt.bfloat16, name="qhat")
        rq_dram = dram.tile([1, S], dt.float32, name="rq_dram")

        QH = _chunks(S, 512)  # [(0,512),(512,585)]
        MH = NH // 2
        ssq = {}
        for qi, (qa, qb) in enumerate(QH):
            ssq[qi] = ps_s.tile([1, 512], dt.float32, tag="ssq", name=f"ssq{qi}")
        wq_view = w_d["wqT"][:].rearrange("(kc p) n -> p kc n", p=128)
        for m in range(NH):
            if m % MH == 0:
                half = m // MH
                wq_sb = wpool.tile([128, NH, MH * 128], dt.bfloat16, tag="wqh",
                                   name=f"wqh{half}")
                nc.sync.dma_start(
                    out=wq_sb[:],
                    in_=wq_view[:, :, half * MH * 128:(half + 1) * MH * 128])
            mo = m % MH
            ps = {}
            for qi, (qa, qb) in enumerate(QH):
                ps[qi] = ps_k.tile([128, 512], dt.float32, tag="psq", name=f"psq{qi}")
                for kc in range(NH):
                    nc.tensor.matmul(ps[qi][:, :qb - qa],
                                     wq_sb[:, kc, mo * 128:(mo + 1) * 128],
                                     xq_sb[:, kc, qa:qb],
                                     start=(kc == 0), stop=(kc == NH - 1))
                sq = evq.tile([128, 512], dt.bfloat16, tag="sqq")
                nc.scalar.activation(sq[:, :qb - qa], ps[qi][:, :qb - qa],
                                     AF.Square, bias=bq_sb[:, m:m + 1])
                nc.scalar.activation(qhat[:, m, qa:qb], ps[qi][:, :qb - qa],
                                     AF.Identity, bias=bqgq_sb[:, m:m + 1],
                                     scale=gq_sb[:, m:m + 1])
                nc.tensor.matmul(ssq[qi][0:1, :qb - qa], ones_key[:],
                                 sq[:, :qb - qa],
                                 start=(m == 0), stop=(m == NH - 1))
        for qi, (qa, qb) in enumerate(QH):
            rt = evq.tile([1, 512], dt.float32, tag="rtq")
            nc.scalar.activation(rt[0:1, :qb - qa], ssq[qi][0:1, :qb - qa],
                                 AF.Sqrt, bias=eps_t[0:1, :], scale=1.0 / DIM)
            nc.vector.reciprocal(rt[0:1, :qb - qa], rt[0:1, :qb - qa])
            nc.sync.dma_start(out=rq_dram[0:1, qa:qb], in_=rt[0:1, :qb - qa])
        rbq = rpool.tile([128, S], dt.float32, name="rbq")
        nc.sync.dma_start(
            out=rbq[:],
            in_=bass.AP(tensor=rq_dram.tensor, offset=rq_dram[0:1, :].offset,
                        ap=[[0, 128], [1, S]]))
        ctq = rpool.tile([128, S], dt.bfloat16, name="ctq")
        stq = rpool.tile([128, S], dt.bfloat16, name="stq")
        nc.vector.tensor_mul(ctq[:], trigq[:, 1, :], rbq[:])
        nc.vector.tensor_mul(stq[:], trigq[:, 0, :], rbq[:])
        for m in range(NH):
            sw = rpool.tile([128, S], dt.bfloat16, tag="swq")
            nc.sync.dma_start(out=sw[0:64, :], in_=qhat[64:128, m, :])
            nc.sync.dma_start(out=sw[64:128, :], in_=qhat[0:64, m, :])
            t1 = rpool.tile([128, S], dt.bfloat16, tag="t1q")
            t2 = rpool.tile([128, S], dt.bfloat16, tag="t2q")
            nc.vector.tensor_mul(t1[:], qhat[:, m, :], ctq[:])
            nc.vector.tensor_mul(t2[:], sw[:], stq[:])
            nc.vector.tensor_add(qrot[:, m, :], t1[:], t2[:])
        qp.close()

        # ================= attention =================
        # per head-pair pg: keys of frame kf are columns [kf*FRAME, ...) of
        # krot; V streamed from v_loc per (kf, chunk).  Scores land in a
        # 4-bank PSUM tile s_t; exp is one ACT op per batch of <=4 chunks.
        # o accumulates in PSUM (passA: q cols [0,512) -- 2 banks; passB:
        # [512,S) -- 1 shared bank), z via ones-matmuls into 1 bank.
        oT_sb = resid.tile([128, NH, S], dt.bfloat16, name="oT_sb")
        JB = 4  # PSUM banks per score tile
        PASSES = _chunks(S, 512)
        att_s = att.enter_context(tc.tile_pool(name="att_s", bufs=1, space="PSUM"))
        att_o = att.enter_context(tc.tile_pool(name="att_o", bufs=1, space="PSUM"))
        att_z = att.enter_context(tc.tile_pool(name="att_z", bufs=1, space="PSUM"))
        att_p = att.enter_context(tc.tile_pool(name="att_p", bufs=3))
        att_m = att.enter_context(tc.tile_pool(name="att_m", bufs=2))

        PB = S - 512  # passB width

        for pg in range(NPG):
            load_pg(pg)
            kr_t, v_t = kr_cache.pop(pg)
            load_pg(pg + 1)

            oA = att_o.tile([128, 2, 512], dt.float32, name="oA")

            # two rounds over query ranges; PSUM o banks are reused across
            # rounds (disjoint group lifetimes -- a (bank, partition-row) can
            # only host one live accumulation group at a time); z double-
            # buffered across rounds
            for pi, (Qa, Qb) in enumerate(PASSES):
                QW = Qb - Qa
                z_t = att_z.tile([128, 512], dt.float32, tag="z", name="z_t")
                # batch geometry per key-frame: small query widths pack
                # several score tiles per PSUM bank so each batch carries
                # enough matmul work to hide the exp round-trip latency
                batches = []
                for kf in range(F):
                    qa = max(Qa, T * first_qf[kf])
                    if qa >= Qb:
                        continue
                    qw = Qb - qa
                    slotw = 128 if qw <= 128 else (256 if qw <= 256 else 512)
                    per_bank = 512 // slotw
                    cap = JB * per_bank
                    units = [(kf, ci, hi, qa)
                             for ci in range(NKCH) for hi in range(2)]
                    for i in range(0, len(units), cap):
                        batches.append((slotw, per_bank, units[i:i + cap]))

                o_first, o_last = {}, {}
                for bi, (slotw, per_bank, b) in enumerate(batches):
                    for j, (kf, ci, hi, qa) in enumerate(b):
                        o_first.setdefault(hi, (bi, j))
                        o_last[hi] = (bi, j)

                def slot_ap(tile_, j, per_bank, slotw, kw, qw):
                    return tile_[:kw, j // per_bank,
                                 (j % per_bank) * slotw:
                                 (j % per_bank) * slotw + qw]

                s_tiles = {}

                def scores(bi):
                    slotw, per_bank, b = batches[bi]
                    s_t = att_s.tile([128, JB, 512], dt.float32, tag="s",
                                     name="s_t")
                    s_tiles[bi] = s_t
                    for j, (kf, ci, hi, qa) in enumerate(b):
                        kw = min(128, FRAME - 128 * ci)
                        nc.tensor.matmul(
                            slot_ap(s_t, j, per_bank, slotw, kw, Qb - qa),
                            kr_t[:, hi,
                                 kf * FRAME + 128 * ci:kf * FRAME + 128 * ci + kw],
                            qrot[:, 2 * pg + hi, qa:Qb],
                            start=True, stop=True)

                def expev(bi):
                    slotw, per_bank, b = batches[bi]
                    kf, ci, hi, qa = b[0]
                    qw = Qb - qa
                    n = len(b)
                    s_t = s_tiles.pop(bi)
                    p_t = att_p.tile([128, JB, 512], dt.bfloat16, tag="p")
                    fb, rem = n // per_bank, n % per_bank
                    base_s, base_p = s_t[:], p_t[:]
                    if fb:
                        ap_s = bass.AP(tensor=base_s.tensor, offset=base_s.offset,
                                       ap=[base_s.ap[0], [512, fb],
                                           [slotw, per_bank], [1, qw]])
                        ap_p = bass.AP(tensor=base_p.tensor, offset=base_p.offset,
                                       ap=[base_p.ap[0], [512, fb],
                                           [slotw, per_bank], [1, qw]])
                        nc.scalar.activation(ap_p, ap_s, AF.Exp,
                                             scale=inv_sqrt_d)
                    if rem:
                        off = fb * 512
                        ap_s = bass.AP(tensor=base_s.tensor,
                                       offset=base_s.offset + off,
                                       ap=[base_s.ap[0], [slotw, rem], [1, qw]])
                        ap_p = bass.AP(tensor=base_p.tensor,
                                       offset=base_p.offset + off,
                                       ap=[base_p.ap[0], [slotw, rem], [1, qw]])
                        nc.scalar.activation(ap_p, ap_s, AF.Exp,
                                             scale=inv_sqrt_d)
                    return p_t

                def pv_z(bi, p_t):
                    slotw, per_bank, b = batches[bi]
                    for j, (kf, ci, hi, qa) in enumerate(b):
                        kw = min(128, FRAME - 128 * ci)
                        qw = Qb - qa
                        nc.tensor.matmul(
                            oA[:, hi, qa - Qa:Qb - Qa],
                            v_t[(kf, ci)][:kw, hi * 128:(hi + 1) * 128],
                            slot_ap(p_t, j, per_bank, slotw, kw, qw),
                            start=o_first[hi] == (bi, j),
                            stop=o_last[hi] == (bi, j))
                        nc.tensor.matmul(
                            z_t[32 * hi:32 * hi + 1, qa - Qa:Qb - Qa],
                            ones_key[:kw, :],
                            slot_ap(p_t, j, per_bank, slotw, kw, qw),
                            start=o_first[hi] == (bi, j),
                            stop=o_last[hi] == (bi, j))

                prev = None
                prev_p = None
                for bi in range(len(batches)):
                    if prev is not None:
                        prev_p = expev(prev)
                    scores(bi)
                    if prev is not None:
                        pv_z(prev, prev_p)
                    prev = bi
                prev_p = expev(prev)
                pv_z(prev, prev_p)

                # 1/Z + eviction for this round (partition-broadcast of
                # 1/z via a DRAM round-trip)
                for hi in range(2):
                    hh = 2 * pg + hi
                    zr = 32 * hi
                    z_sb = att_m.tile([128, 512], dt.float32, tag="zsb",
                                      name="zsb")
                    z_dram = dram.tile([1, 512], dt.float32, tag="zdram",
                                       bufs=2, name="zdram")
                    nc.scalar.activation(z_sb[zr:zr + 1, :QW],
                                         z_t[zr:zr + 1, :QW], AF.Copy)
                    nc.vector.reciprocal(z_sb[zr:zr + 1, :QW],
                                         z_sb[zr:zr + 1, :QW])
                    nc.sync.dma_start(out=z_dram[0:1, :QW],
                                      in_=z_sb[zr:zr + 1, :QW])
                    izb = att_m.tile([128, 512], dt.float32, tag="izb",
                                     name="izb")
                    nc.sync.dma_start(
                        out=izb[:, :QW],
                        in_=bass.AP(tensor=z_dram.tensor,
                                    offset=z_dram[0:1, :].offset,
                                    ap=[[0, 128], [1, QW]]))
                    if debug:
                        nc.sync.dma_start(out=zdbg_d[hh:hh + 1, Qa:Qb],
                                          in_=izb[0:1, :QW])
                    nc.vector.tensor_mul(oT_sb[:, hh, Qa:Qb],
                                         oA[:, hi, :QW], izb[:, :QW])
        att.close()
        if debug:
            nc.sync.dma_start(out=kdbg_d[:], in_=k_loc[:])
            nc.sync.dma_start(out=vdbg_d[:], in_=v_loc[:])
            nc.sync.dma_start(out=qdbg_d[:],
                              in_=qrot[:].rearrange("p m s -> p (m s)"))
            nc.sync.dma_start(out=odbg_d[:],
                              in_=oT_sb[:].rearrange("p m s -> p (m s)"))

        # ---------------- O projection ----------------
        bo_bc = const.tile([128, DIM], dt.float32)
        nc.sync.dma_start(
            out=bo_bc[:],
            in_=bass.AP(tensor=bo_d[:].tensor, offset=bo_d[:].offset,
                        ap=[[0, 128]] + bo_d[:].ap[1:]),
        )
        wpool = ctx.enter_context(tc.tile_pool(name="w_o", bufs=3))
        pspool = ctx.enter_context(
            tc.tile_pool(name="ps_o", bufs=len(TOKCH) + 1, space="PSUM"))
        evpool = ctx.enter_context(tc.tile_pool(name="ev_o", bufs=3))
        for sl in range(NSL):
            ps = {}
            for ti in range(len(TOKCH)):
                ps[ti] = pspool.tile([128, SLICE], dt.float32, tag="ops",
                                     name=f"ops{ti}")
            for m in range(NH):
                wt = wpool.tile([128, SLICE], dt.bfloat16, tag="wo")
                nc.sync.dma_start(
                    out=wt[:],
                    in_=w_d["woT"][m * 128:(m + 1) * 128,
                                   sl * SLICE:(sl + 1) * SLICE])
                for ti, (ta, tb) in enumerate(TOKCH):
                    nc.tensor.matmul(ps[ti][:tb - ta, :], oT_sb[:, m, ta:tb],
                                     wt[:], start=(m == 0), stop=(m == NH - 1))
            for ti, (ta, tb) in enumerate(TOKCH):
                tw = tb - ta
                ot = evpool.tile([128, SLICE], dt.float32, tag="oev")
                nc.vector.tensor_add(ot[:tw, :], ps[ti][:tw, :],
                                     bo_bc[:tw, sl * SLICE:(sl + 1) * SLICE])
                nc.sync.dma_start(
                    out=out_d[ta:tb, sl * SLICE:(sl + 1) * SLICE],
                    in_=ot[:tw, :])

    if cap_waits:
        _cap_sync_waits(nc, mybir)
    _BUILD_CACHE[key] = nc
    return nc


def _cap_sync_waits(nc, mybir, cap=1):
    """Walrus engine-instruction structs only have a limited number of sync
    wait slots.  Hoist excess waits onto InstNoOp carriers placed immediately
    before the instruction on the same engine stream."""
    exempt = (mybir.InstNoOp, mybir.InstEventSemaphore,
              mybir.InstAllEngineBarrier)
    for f in nc.m.functions:
        for bb in f.blocks:
            out = []
            changed = False
            for inst in bb.instructions:
                si = inst.sync_info
                if (si is None or len(si.on_wait) <= cap
                        or isinstance(inst, exempt)):
                    out.append(inst)
                    continue
                waits = list(si.on_wait)
                keep, excess = waits[:cap], waits[cap:]
                while excess:
                    batch, excess = excess[:cap], excess[cap:]
                    out.append(mybir.InstNoOp(
                        name=f"{inst.name}-w{len(out)}",
                        engine=inst.engine,
                        bass_nofuse=True,
                        sync_info=mybir.SyncInfo(on_wait=batch, on_update=[]),
                    ))
                inst.sync_info = mybir.SyncInfo(on_wait=keep,
                                                on_update=list(si.on_update))
                out.append(inst)
                changed = True
            if changed:
                bb.instructions = out


# ---------------------------------------------------------------------------
# host side
# ---------------------------------------------------------------------------
def _perm(NH):
    p = np.empty(NH * D, np.int64)
    for hh in range(NH):
        base = hh * D
        for j in range(D // 2):
            p[base + j] = base + 2 * j
            p[base + D // 2 + j] = base + 2 * j + 1
    return p


def _angles(freqs, idx, FRAME, w, start_frame, c0, c1, c):
    """Phase-shifted RoPE angle tables [128, len(idx)] for tokens `idx`."""
    fr = idx // FRAME
    rem = idx % FRAME
    hh_i = rem // w
    ww_i = rem % w
    n = len(idx)
    ang = np.empty((c, n), np.float32)
    ang[:c0, :] = freqs[start_frame + fr][:, :c0].T
    ang[c0:c0 + c1, :] = freqs[hh_i][:, c0:c0 + c1].T
    ang[c0 + c1:, :] = freqs[ww_i][:, c0 + c1:c].T

    def wrap(a):
        a = np.asarray(a, np.float64)
        return (a - 2 * np.pi * np.round(a / (2 * np.pi))).astype(np.float32)

    # top half encodes -sin via the (ang + pi) phase shift
    angS = np.ascontiguousarray(
        np.concatenate([wrap(ang + np.pi), wrap(ang)], 0), np.float32)
    angC = np.ascontiguousarray(
        np.concatenate([wrap(ang + np.pi / 2), wrap(ang + np.pi / 2)], 0),
        np.float32)
    return angS, angC


def _host_inputs(x, freqs, Wq, bq, Wk, bk, Wv, bv, Wo, bo, gq, gk,
                 f, h, w, num_heads, local_attn_size, sink_size, start_frame):
    NH = num_heads
    DIM = NH * D
    FRAME = h * w
    assert FRAME % NC == 0
    T = FRAME // NC
    S = f * T
    L = f * FRAME
    perm = _perm(NH)

    def bf(a):
        return np.ascontiguousarray(a, dtype=np.float32).astype(BF16)

    wqT = bf(Wq[perm].T)
    wkT = bf(Wk[perm].T)
    wvT = bf(Wv.T)
    woT = bf(Wo.T)

    def chunkmajor(a):
        return np.asarray(a, np.float32)[perm].reshape(NH, D).T
    bias_pack = np.ascontiguousarray(np.concatenate(
        [chunkmajor(bq), chunkmajor(gq), chunkmajor(bq) * chunkmajor(gq),
         chunkmajor(bk), chunkmajor(gk), chunkmajor(bk) * chunkmajor(gk)],
        axis=1), np.float32)
    bv_r = bf(bv.reshape(1, DIM))
    bo_r = np.ascontiguousarray(bo.reshape(1, DIM), np.float32)

    c = D // 2
    c1 = c // 3
    c0 = c - 2 * c1
    freqs = np.asarray(freqs, np.float32)

    xT_full = bf(np.asarray(x[0], np.float32).T)
    all_idx = np.arange(L)
    angS, angC = _angles(freqs, all_idx, FRAME, w, start_frame, c0, c1, c)

    in_maps = []
    tok_idx = []
    for core in range(NC):
        idx = np.concatenate(
            [fr * FRAME + T * core + np.arange(T) for fr in range(f)])
        tok_idx.append(idx)
        xTq = bf(np.asarray(x[0], np.float32)[idx].T)
        aqS, aqC = _angles(freqs, idx, FRAME, w, start_frame, c0, c1, c)
        in_maps.append({
            "xT": xT_full, "xTq": xTq,
            "wqT": wqT, "wkT": wkT, "wvT": wvT, "woT": woT,
            "bias_pack": bias_pack,
            "bv_r": bv_r, "bo_r": bo_r,
            "angS": angS, "angC": angC, "aqS": aqS, "aqC": aqC,
        })
    return in_maps, tok_idx, T, S


def _allowed(f, local_attn_size, sink_size):
    return [
        [kf for kf in range(f)
         if kf <= qf and (qf - kf < local_attn_size or kf < sink_size)]
        for qf in range(f)
    ]


def kernel(x, freqs, Wq, bq, Wk, bk, Wv, bv, Wo, bo, gq, gk,
           f, h, w, num_heads, local_attn_size, sink_size, start_frame,
           _trace=False):
    from concourse.bass_utils import run_bass_kernel_spmd

    f = int(f); h = int(h); w = int(w)
    num_heads = int(num_heads)
    local_attn_size = int(local_attn_size)
    sink_size = int(sink_size)
    start_frame = int(start_frame)

    x = np.asarray(x)
    B, L, DIM = x.shape
    assert B == 1 and DIM == num_heads * D

    allowed = _allowed(f, local_attn_size, sink_size)
    in_maps, tok_idx, T, S = _host_inputs(
        x, freqs, Wq, bq, Wk, bk, Wv, bv, Wo, bo, gq, gk,
        f, h, w, num_heads, local_attn_size, sink_size, start_frame)
    nc = build_program(num_heads, f, T, allowed)
    res = run_bass_kernel_spmd(nc, in_maps, core_ids=list(range(NC)),
                               trace=_trace)
    out = np.empty((1, L, DIM), np.float32)
    for core in range(NC):
        out[0, tok_idx[core]] = res.results[core]["out"]
    if _trace:
        kernel._last_results = res
    return out
